# revision 1
# baseline (speedup 1.0000x reference)
"""GAT (2-layer, DGL-style) Trainium2 Bass kernel, 8-core SPMD.

Strategy:
- Shard by dst-node range: core k owns nodes [2048k, 2048k+2048) and all edges
  pointing into them (host-sorted by dst, padded per 128-dst block).
- Node phase (per core, own nodes): h = [aa_emb[ref_aa]*galt | feat] built
  feature-major (hT), one fused f16 matmul produces ft/el/er/res columns.
- Gather tables [16384+, 256] f16 rows = [ft(128) | el(2) | pad]: each core's
  slice replicated 8x into a staged buffer, exchanged with AllToAll -> full
  table in every core's DRAM.
- Edge phase per 128-dst block: dma_gather rows by src (512B f16 rows),
  er expanded per edge via transposed-one-hot matmuls, e=el+er, leaky, exp,
  X = [ft*ez | ez] scaled with broadcast APs, scatter-add via one-hot matmul
  into PSUM (numerator+denominator in one), normalize, +residual, ELU.
- Layer 1 uses the linearity trick: gather h1 (128 cols) instead of ft1 (512),
  apply W1 per head after aggregation. Head-mean/residual/bias folded into
  host-prepared weights. Graph mean + MLP readout on-device; host assembles
  the [32, 2] output from per-core [2, 4] logits.
"""

import math
import numpy as np

import concourse.bass as bass
import concourse.tile as tile
from concourse import bacc, mybir
from concourse.bass_utils import run_bass_kernel_spmd
from concourse.masks import make_identity

# Problem constants (hardcoded per harness contract)
N = 16384
E = 262144
G = 32
NPG = 512
D_FEAT = 256
AA_DIM = 64
N_LABELS = 21
H = 2
O0 = 64
O1 = 256
NEG = 0.2

NC = 8
NNODE = N // NC          # 2048 nodes per core
P = 128
NBLK = NNODE // P        # 16 dst blocks per core
NTILE = NNODE // P       # 16 node tiles per core
GPC = G // NC            # 4 graphs per core
TROW = 256               # f16 table row elements (512B)
DUMMY = N                # dummy table row index
TBL_ROWS = N + P         # table rows incl dummy + pad

f32 = mybir.dt.float32
f16 = mybir.dt.float16
f8 = mybir.dt.float8e4
i16 = mybir.dt.int16


def _wrap_idx(arr):
    """[NI] int array -> [128, NI//16] int16 replicated across 8 groups."""
    ni = arr.shape[0]
    w = arr.astype(np.int16).reshape(ni // 16, 16).T
    return np.tile(w, (8, 1)).copy()


def _host_prep(inputs):
    feat = np.asarray(inputs['feat'], np.float32)
    aa_emb = np.asarray(inputs['aa_emb'], np.float32)
    W0 = np.asarray(inputs['W0'], np.float32)
    al0 = np.asarray(inputs['al0'], np.float32)
    ar0 = np.asarray(inputs['ar0'], np.float32)
    res0 = np.asarray(inputs['res0'], np.float32)
    b0 = np.asarray(inputs['b0'], np.float32)
    W1 = np.asarray(inputs['W1'], np.float32)
    al1 = np.asarray(inputs['al1'], np.float32)
    ar1 = np.asarray(inputs['ar1'], np.float32)
    res1 = np.asarray(inputs['res1'], np.float32)
    b1 = np.asarray(inputs['b1'], np.float32)
    mW0 = np.asarray(inputs['mW0'], np.float32)
    mb0 = np.asarray(inputs['mb0'], np.float32)
    mW1 = np.asarray(inputs['mW1'], np.float32)
    mb1 = np.asarray(inputs['mb1'], np.float32)
    mW2 = np.asarray(inputs['mW2'], np.float32)
    mb2 = np.asarray(inputs['mb2'], np.float32)
    ref_aa = np.asarray(inputs['ref_aa'], np.int64)
    alt_aa = np.asarray(inputs['alt_aa'], np.int64)
    src = np.asarray(inputs['src'], np.int64)
    dst = np.asarray(inputs['dst'], np.int64)
    graph_id = np.asarray(inputs['graph_id'], np.int64)

    # ---- weights ----
    A0 = np.stack([W0[:, 64 * h:64 * h + 64] @ al0[h] for h in range(H)], 1)  # [320,2]
    B0 = np.stack([W0[:, 64 * h:64 * h + 64] @ ar0[h] for h in range(H)], 1)
    din0 = AA_DIM + D_FEAT
    wcat0 = np.zeros((din0 + 1, 260), np.float32)
    wcat0[:din0, 0:128] = W0
    wcat0[:din0, 128:130] = A0
    wcat0[:din0, 130:132] = B0
    wcat0[:din0, 132:260] = res0
    wcat0[din0, 132:260] = b0

    A1 = np.stack([W1[:, 256 * h:256 * h + 256] @ al1[h] for h in range(H)], 1)  # [128,2]
    B1 = np.stack([W1[:, 256 * h:256 * h + 256] @ ar1[h] for h in range(H)], 1)
    w1cat = np.zeros((129, 4), np.float32)
    w1cat[:128, 0:2] = A1
    w1cat[:128, 2:4] = B1
    w1hm = np.concatenate([W1[:, 0:256], W1[:, 256:512]], 1) * 0.5  # [128,512]
    res1m = np.zeros((129, 256), np.float32)
    res1m[:128] = (res1[:, 0:256] + res1[:, 256:512]) * 0.5
    res1m[128] = (b1[0:256] + b1[256:512]) * 0.5

    mw0p = (mW0 / np.float32(NPG)).astype(np.float32)  # [256,128], folds mean-readout

    galt = aa_emb[alt_aa]  # [32, 64]

    # ---- edges ----
    per_core = []
    s_needed = 1
    for k in range(NC):
        lo = k * NNODE
        m = (dst >= lo) & (dst < lo + NNODE)
        es = src[m]
        ed = dst[m]
        order = np.argsort(ed, kind='stable')
        es, ed = es[order], ed[order]
        blk = (ed - lo) // P
        counts = np.bincount(blk, minlength=NBLK)
        s_needed = max(s_needed, int(math.ceil(counts.max() / P)))
        per_core.append((es, ed, blk, counts, lo))
    S = s_needed  # uniform subtiles per block across all cores

    cores = []
    for k in range(NC):
        es, ed, blk, counts, lo = per_core[k]
        NIB = S * P
        src_pad = np.full((NBLK, NIB), DUMMY, np.int64)
        dst_pad = np.zeros((NBLK, NIB), np.int64)
        valid = np.zeros((NBLK, NIB), bool)
        off = 0
        for b in range(NBLK):
            c = counts[b]
            src_pad[b, :c] = es[off:off + c]
            dst_pad[b, :c] = (ed[off:off + c] - lo) % P
            valid[b, :c] = True
            off += c
        srcw_b = [_wrap_idx(src_pad[b]) for b in range(NBLK)]
        srcw = np.stack([np.concatenate([srcw_b[2 * j], srcw_b[2 * j + 1]], 1)
                         for j in range(NBLK // 2)])  # [NBLK/2,128,2*NIB//16]
        dl = dst_pad.reshape(NBLK, S, P)
        v = valid.reshape(NBLK, S, P)
        ohb = (dl[..., None] == np.arange(P)) & v[..., None]           # [NBLK,S,e,d]
        import ml_dtypes
        oh = ohb.transpose(0, 2, 1, 3).astype(ml_dtypes.float8_e4m3).copy()   # [NBLK,e,S,d]
        ohT = ohb.transpose(0, 3, 1, 2).astype(ml_dtypes.float8_e4m3).copy()  # [NBLK,d,S,e]

        rng = slice(lo, lo + NNODE)
        featT16 = np.ascontiguousarray(feat[rng].T).astype(np.float16)  # [256,2048]
        galt_pt = galt[graph_id[rng]].reshape(NTILE, P, AA_DIM) \
            .transpose(1, 0, 2).astype(np.float16).copy()               # [128,16,64]
        aaidx = _wrap_idx(ref_aa[rng])                                  # [128,128]

        cores.append(dict(
            featT16=featT16,
            galt=galt_pt,
            aaidx=aaidx,
            srcw=srcw.astype(np.int16),
            oh=oh, ohT=ohT,
            wcat0=wcat0.astype(np.float16),
            w1cat=w1cat.astype(np.float16),
            w1hm=w1hm.astype(np.float16),
            res1m=res1m.astype(np.float16),
            aa_emb=aa_emb,
            mw0p=mw0p, mb0=mb0.reshape(-1, 1),
            mw1=mW1, mb1=mb1.reshape(-1, 1),
            mw2=mW2, mb2=mb2.reshape(-1, 1),
        ))
    return cores, S


def _build(nc_b, S, no_collectives=False):
    """Emit the kernel for one (shared) 8-core program."""
    NIB = S * P

    inp = {}

    def di(name, shape, dt):
        t = nc_b.dram_tensor(name, list(shape), dt, kind="ExternalInput")
        inp[name] = t.ap()
        return inp[name]

    featT16 = di('featT16', (D_FEAT, NNODE), f16)
    galt = di('galt', (P, NTILE, AA_DIM), f16)
    aaidx = di('aaidx', (P, NNODE // 16), i16)
    aa_emb = di('aa_emb', (N_LABELS, AA_DIM), f32)
    srcw = di('srcw', (NBLK // 2, P, 2 * NIB // 16), i16)
    oh_in = di('oh', (NBLK, P, S, P), f8)
    ohT_in = di('ohT', (NBLK, P, S, P), f8)
    wcat0 = di('wcat0', (321, 260), f16)
    w1cat = di('w1cat', (129, 4), f16)
    w1hm = di('w1hm', (P, 512), f16)
    res1m = di('res1m', (129, 256), f16)
    mw0p = di('mw0p', (256, 128), f32)
    mb0 = di('mb0', (128, 1), f32)
    mw1 = di('mw1', (128, 64), f32)
    mb1 = di('mb1', (64, 1), f32)
    mw2 = di('mw2', (64, 2), f32)
    mb2 = di('mb2', (2, 1), f32)

    out_t = nc_b.dram_tensor('logitsT', [2, GPC], f32, kind="ExternalOutput")
    out = out_t.ap()

    with tile.TileContext(nc_b) as tc:
        with (
            tc.tile_pool(name="sb", bufs=1) as sb,
            tc.tile_pool(name="sbw", bufs=2) as sbw,
            tc.tile_pool(name="pp", bufs=3, space="PSUM") as pp,
            tc.tile_pool(name="ppa", bufs=2, space="PSUM") as ppa,
            tc.tile_pool(name="dr", bufs=1, space="DRAM") as dr,
        ):
            nc = nc_b

            staged0 = dr.tile([N, TROW], f16)
            full0 = dr.tile([TBL_ROWS, TROW], f16)
            staged1 = dr.tile([N, TROW], f16)
            full1 = dr.tile([TBL_ROWS, TROW], f16)

            ident = sb.tile([P, P], f32)
            make_identity(nc, ident[:])
            ident16 = sb.tile([P, P], f16)
            nc.vector.tensor_copy(out=ident16[:], in_=ident[:])
            oh_all = sb.tile([P, NBLK, S, P], f8)
            ohT_all = sb.tile([P, NBLK, S, P], f8)
            for bq in range(NBLK):
                eng = nc.sync if bq % 2 == 0 else nc.scalar
                eng.dma_start(out=oh_all[:, bq, :, :], in_=oh_in[bq])
                eng.dma_start(out=ohT_all[:, bq, :, :], in_=ohT_in[bq])

            # ---------------- node phase L0 ----------------
            nod_cm = tc.tile_pool(name="nod", bufs=1)
            nod = nod_cm.__enter__()
            # hT chunks: rows 0..64 haaT, 64..320 featT, 320 ones
            hta = nod.tile([P, NNODE], f16, tag="hta")   # rows 0..128 of hT
            htb = nod.tile([P, NNODE], f16, tag="htb")   # rows 128..256
            htc = nod.tile([65, NNODE], f16, tag="htc")  # rows 256..321 (incl ones row)
            nc.sync.dma_start(out=hta[64:128, :], in_=featT16[0:64, :])
            nc.sync.dma_start(out=htb[:, :], in_=featT16[64:192, :])
            nc.sync.dma_start(out=htc[0:64, :], in_=featT16[192:256, :])
            nc.vector.memset(htc[64:65, :], 1.0)

            aaidx_t = nod.tile([P, NNODE // 16], i16)
            nc.sync.dma_start(out=aaidx_t[:], in_=aaidx[:])
            aag = nod.tile([P, NTILE, AA_DIM], f32)
            nc.gpsimd.dma_gather(out_ap=aag[:], in_ap=aa_emb[:], idxs_ap=aaidx_t[:],
                                 num_idxs=NNODE, num_idxs_reg=NNODE,
                                 elem_size=AA_DIM, single_packet=False)
            galt_t = nod.tile([P, NTILE, AA_DIM], f16)
            nc.sync.dma_start(out=galt_t[:], in_=galt[:])
            haa = nod.tile([P, NTILE, AA_DIM], f32)
            nc.vector.tensor_tensor(out=haa[:], in0=aag[:], in1=galt_t[:],
                                    op=mybir.AluOpType.mult)
            for t in range(NTILE):
                tp = pp.tile([AA_DIM, P], f32, space="PSUM", tag="w")
                nc.tensor.transpose(out=tp[:], in_=haa[:, t, :], identity=ident[:])
                nc.vector.tensor_copy(out=hta[0:64, t * P:(t + 1) * P], in_=tp[:])

            w0a = nod.tile([P, 260], f16)
            w0b = nod.tile([P, 260], f16)
            w0c = nod.tile([65, 260], f16)
            nc.sync.dma_start(out=w0a[:], in_=wcat0[0:128, :])
            nc.sync.dma_start(out=w0b[:], in_=wcat0[128:256, :])
            nc.sync.dma_start(out=w0c[:], in_=wcat0[256:321, :])

            er0_own = sb.tile([P, NTILE, H], f32)
            res0_own = sb.tile([P, NTILE, P], f32)
            t0rows = sb.tile([P, NTILE, TROW], f16, tag="trows")
            nc.vector.memset(t0rows[:], 0)

            for t in range(NTILE):
                ps0 = pp.tile([P, 260], f32, space="PSUM", tag="w")
                cs = slice(t * P, (t + 1) * P)
                nc.tensor.matmul(out=ps0[:], lhsT=hta[:, cs], rhs=w0a[:], start=True, stop=False)
                nc.tensor.matmul(out=ps0[:], lhsT=htb[:, cs], rhs=w0b[:], start=False, stop=False)
                nc.tensor.matmul(out=ps0[:], lhsT=htc[:, cs], rhs=w0c[:], start=False, stop=True)
                nc.vector.tensor_copy(out=t0rows[:, t, 0:130], in_=ps0[:, 0:130])
                nc.vector.tensor_copy(out=er0_own[:, t, :], in_=ps0[:, 130:132])
                nc.vector.tensor_copy(out=res0_own[:, t, :], in_=ps0[:, 132:260])
            # staged layout: rows r = t*128 + p -> AP [p, (t, d)]
            st0v = staged0[:].rearrange("(t p) d -> p t d", p=P)
            for c in range(NC):
                nc.sync.dma_start(out=st0v[:, c * NTILE:(c + 1) * NTILE, :],
                                  in_=t0rows[:])
            nod_cm.__exit__(None, None, None)

            if no_collectives:
                nc.gpsimd.dma_start(out=full0[0:N, :], in_=staged0[:])
            else:
                nc.gpsimd.collective_compute(
                    "AllToAll", mybir.AluOpType.bypass,
                    replica_groups=[list(range(NC))],
                    ins=[staged0.opt()], outs=[full0[0:N, :].opt()])
            zrow = sb.tile([1, TROW], f16)
            nc.vector.memset(zrow[:], 0)
            nc.sync.dma_start(out=full0[DUMMY:DUMMY + 1, :], in_=zrow[:])

            # ---------------- shared edge-phase helper ----------------
            h1_own = sb.tile([P, NTILE, P], f32)

            def edge_pair(j, full_tbl, er_own_t, layer):
                """Process dst blocks (2j, 2j+1); returns two rst tiles."""
                XW = 130 if layer == 0 else 258
                FW = 64 if layer == 0 else 128
                S2 = 2 * S
                it = sbw.tile([P, 2 * NIB // 16], i16, tag="it")
                nc.scalar.dma_start(out=it[:], in_=srcw[j])
                g = sbw.tile([P, S2, TROW], f16, tag="g")
                nc.gpsimd.dma_gather(out_ap=g[:], in_ap=full_tbl[:], idxs_ap=it[:],
                                     num_idxs=2 * NIB, num_idxs_reg=2 * NIB,
                                     elem_size=TROW, single_packet=False)
                er16 = sbw.tile([P, 2, H], f16, tag="er16")
                nc.vector.tensor_copy(out=er16[:], in_=er_own_t[:, 2 * j:2 * j + 2, :])
                pser = pp.tile([P, S2, H], f32, space="PSUM", tag="pser")
                for half in range(2):
                    ohTb = ohT_all[:, 2 * j + half]
                    for s in range(S):
                        nc.tensor.matmul(out=pser[:, half * S + s, :],
                                         lhsT=ohTb[:, s, :],
                                         rhs=er16[:, half, :], start=True, stop=True)
                # e = el + er ; leaky ; exp
                el_view = g[:, :, 128:130]
                ez = sbw.tile([P, S2, H], f32, tag="ez")
                nc.vector.tensor_tensor(out=ez[:], in0=el_view, in1=pser[:],
                                        op=mybir.AluOpType.add)
                nc.vector.scalar_tensor_tensor(out=ez[:], in0=ez[:], scalar=NEG,
                                               in1=ez[:], op0=mybir.AluOpType.mult,
                                               op1=mybir.AluOpType.max)
                ez16 = sbw.tile([P, S2, H], f16, tag="ez16")
                nc.scalar.activation(out=ez16[:], in_=ez[:],
                                     func=mybir.ActivationFunctionType.Exp)
                # X = [ft_h * ez_h | ez], one tile per half
                rsts = []
                for half in range(2):
                    X = sbw.tile([P, S, XW], f16, tag="X")
                    hs = slice(half * S, half * S + S)
                    if layer == 0:
                        g_heads = g[:, hs, 0:H * FW].rearrange("p s (h f) -> p s h f", h=H)
                    else:
                        gv = g[:, hs, :]
                        g_heads = bass.AP(gv.tensor, gv.offset,
                                          [[S2 * TROW, P], [TROW, S], [0, H], [1, FW]])
                    ezv = ez16[:, hs, :]
                    ez_b = bass.AP(ezv.tensor, ezv.offset,
                                   [[S2 * H, P], [H, S], [1, H], [0, FW]])
                    x_heads = X[:, :, 0:H * FW].rearrange("p s (h f) -> p s h f", h=H)
                    nc.vector.tensor_tensor(out=x_heads, in0=g_heads, in1=ez_b,
                                            op=mybir.AluOpType.mult)
                    nc.vector.tensor_copy(out=X[:, :, XW - 2:XW], in_=ez16[:, hs, :])
                    ohb = oh_all[:, 2 * j + half]
                    acc = ppa.tile([P, XW], f32, space="PSUM", tag="acc")
                    for s in range(S):
                        nc.tensor.matmul(out=acc[:], lhsT=ohb[:, s, :],
                                         rhs=X[:, s, :],
                                         start=(s == 0), stop=(s == S - 1))
                    den = sbw.tile([P, H], f32, tag="den")
                    nc.vector.reciprocal(out=den[:], in_=acc[:, XW - 2:XW])
                    rst = sbw.tile([P, H * FW], f32, tag="rst")
                    denv = den[:]
                    den_b = bass.AP(denv.tensor, denv.offset,
                                    [[H, P], [1, H], [0, FW]])
                    nc.vector.tensor_tensor(
                        out=rst[:].rearrange("p (h f) -> p h f", h=H),
                        in0=acc[:, 0:H * FW].rearrange("p (h f) -> p h f", h=H),
                        in1=den_b, op=mybir.AluOpType.mult)
                    rsts.append(rst)
                return rsts

            # ---------------- edge phase L0 + epilogue (+ folded node L1) ----------
            ht1 = sb.tile([P, NNODE], f16)
            ones1 = sb.tile([1, NNODE], f16)
            nc.vector.memset(ones1[:], 1.0)
            w1a = sb.tile([P, 4], f16)
            w1b = sb.tile([1, 4], f16)
            nc.sync.dma_start(out=w1a[:], in_=w1cat[0:128, :])
            nc.sync.dma_start(out=w1b[:], in_=w1cat[128:129, :])
            er1_own = sb.tile([P, NTILE, H], f32)
            t1rows = sb.tile([P, NTILE, TROW], f16, tag="trows")
            nc.vector.memset(t1rows[:], 0)
            for j in range(NBLK // 2):
                rsts = edge_pair(j, full0, er0_own, 0)
                for half in range(2):
                    b = 2 * j + half
                    rst = rsts[half]
                    x = sbw.tile([P, P], f32, tag="h1x")
                    nc.vector.tensor_add(out=x[:], in0=rst[:], in1=res0_own[:, b, :])
                    # ELU
                    mn = sbw.tile([P, P], f32, tag="mn")
                    nc.vector.tensor_scalar_min(out=mn[:], in0=x[:], scalar1=0.0)
                    nc.scalar.activation(out=mn[:], in_=mn[:],
                                         func=mybir.ActivationFunctionType.Exp)
                    mx = sbw.tile([P, P], f32, tag="mx")
                    nc.vector.tensor_scalar_max(out=mx[:], in0=x[:], scalar1=0.0)
                    nc.vector.scalar_tensor_tensor(out=h1_own[:, b, :], in0=mn[:],
                                                   scalar=-1.0, in1=mx[:],
                                                   op0=mybir.AluOpType.add,
                                                   op1=mybir.AluOpType.add)
                    # folded node-phase L1 for this tile
                    cs = slice(b * P, (b + 1) * P)
                    tp1 = pp.tile([P, P], f32, space="PSUM", tag="w")
                    nc.tensor.transpose(out=tp1[:], in_=h1_own[:, b, :], identity=ident[:])
                    nc.vector.tensor_copy(out=ht1[:, cs], in_=tp1[:])
                    ps1 = pp.tile([P, 4], f32, space="PSUM", tag="w")
                    nc.tensor.matmul(out=ps1[:], lhsT=ht1[:, cs], rhs=w1a[:], start=True, stop=False)
                    nc.tensor.matmul(out=ps1[:], lhsT=ones1[:, cs], rhs=w1b[:], start=False, stop=True)
                    nc.vector.tensor_copy(out=t1rows[:, b, 0:128], in_=h1_own[:, b, :])
                    nc.vector.tensor_copy(out=t1rows[:, b, 128:130], in_=ps1[:, 0:2])
                    nc.vector.tensor_copy(out=er1_own[:, b, :], in_=ps1[:, 2:4])

            # ---------------- node phase L1 (folded per-tile above) ----------------
            st1v = staged1[:].rearrange("(t p) d -> p t d", p=P)
            for c in range(NC):
                nc.sync.dma_start(out=st1v[:, c * NTILE:(c + 1) * NTILE, :],
                                  in_=t1rows[:])

            if no_collectives:
                nc.gpsimd.dma_start(out=full1[0:N, :], in_=staged1[:])
            else:
                nc.gpsimd.collective_compute(
                    "AllToAll", mybir.AluOpType.bypass,
                    replica_groups=[list(range(NC))],
                    ins=[staged1.opt()], outs=[full1[0:N, :].opt()])
            nc.sync.dma_start(out=full1[DUMMY:DUMMY + 1, :], in_=zrow[:])

            # ---------------- edge phase L1 + readout ----------------
            wh = sb.tile([P, 512], f16)
            nc.sync.dma_start(out=wh[:], in_=w1hm[:])
            r1a = sb.tile([P, 256], f16)
            r1b = sb.tile([1, 256], f16)
            nc.sync.dma_start(out=r1a[:], in_=res1m[0:128, :])
            nc.sync.dma_start(out=r1b[:], in_=res1m[128:129, :])
            onescol = sb.tile([P, 1], f16)
            nc.vector.memset(onescol[:], 1.0)
            hg = sb.tile([1, GPC, 256], f32)
            nc.vector.memset(hg[:], 0)

            for j in range(NBLK // 2):
              aggs = edge_pair(j, full1, er1_own, 1)
              for half in range(2):
                b = 2 * j + half
                agg = aggs[half]   # [P, 256] f32 normalized
                agg16 = sbw.tile([P, H, P], f16, tag="agg16")
                nc.vector.tensor_copy(out=agg16[:], in_=agg[:].rearrange("p (h f) -> p h f", h=H))
                aggT = sbw.tile([P, H, P], f16, tag="aggT")
                for h in range(H):
                    tpa = pp.tile([P, P], f16, space="PSUM", tag="w")
                    nc.tensor.transpose(out=tpa[:], in_=agg16[:, h, :], identity=ident16[:])
                    nc.vector.tensor_copy(out=aggT[:, h, :], in_=tpa[:])
                ps2 = pp.tile([P, 256], f32, space="PSUM", tag="w")
                cs = slice(b * P, (b + 1) * P)
                nc.tensor.matmul(out=ps2[:], lhsT=aggT[:, 0, :], rhs=wh[:, 0:256],
                                 start=True, stop=False)
                nc.tensor.matmul(out=ps2[:], lhsT=aggT[:, 1, :], rhs=wh[:, 256:512],
                                 start=False, stop=False)
                nc.tensor.matmul(out=ps2[:], lhsT=ht1[:, cs], rhs=r1a[:],
                                 start=False, stop=False)
                nc.tensor.matmul(out=ps2[:], lhsT=ones1[:, cs], rhs=r1b[:],
                                 start=False, stop=True)
                # ELU
                mn = sbw.tile([P, 256], f32, tag="mn2")
                nc.vector.tensor_scalar_min(out=mn[:], in0=ps2[:], scalar1=0.0)
                nc.scalar.activation(out=mn[:], in_=mn[:],
                                     func=mybir.ActivationFunctionType.Exp)
                mx = sbw.tile([P, 256], f32, tag="mx2")
                nc.vector.tensor_scalar_max(out=mx[:], in0=ps2[:], scalar1=0.0)
                h2 = sbw.tile([P, 256], f16, tag="h2")
                nc.vector.scalar_tensor_tensor(out=h2[:], in0=mn[:], scalar=-1.0,
                                               in1=mx[:], op0=mybir.AluOpType.add,
                                               op1=mybir.AluOpType.add)
                gidx = b // (NBLK // GPC)
                pblk = pp.tile([1, 256], f32, space="PSUM", tag="w")
                nc.tensor.matmul(out=pblk[:], lhsT=onescol[:], rhs=h2[:],
                                 start=True, stop=True)
                nc.vector.tensor_add(out=hg[:, gidx, :],
                                     in0=hg[:, gidx, :], in1=pblk[:])

            # readout MLP (feature-major): transpose hg [1, GPC, 256] -> hgT [256c, GPC]
            hgT = sb.tile([P, 2, GPC], f32)
            for gi in range(GPC):
                for c in range(2):
                    tph = pp.tile([P, 1], f32, space="PSUM", tag="w")
                    nc.tensor.transpose(out=tph[:], in_=hg[:, gi, c * P:(c + 1) * P],
                                        identity=ident[0:1, 0:1])
                    nc.vector.tensor_copy(out=hgT[:, c, gi:gi + 1], in_=tph[:])
            mw0_t = sb.tile([P, 2, P], f32)
            nc.sync.dma_start(out=mw0_t[:, 0, :], in_=mw0p[0:128, :])
            nc.sync.dma_start(out=mw0_t[:, 1, :], in_=mw0p[128:256, :])
            mb0_t = sb.tile([P, 1], f32)
            nc.sync.dma_start(out=mb0_t[:], in_=mb0[:])
            px1 = pp.tile([P, GPC], f32, space="PSUM", tag="w")
            nc.tensor.matmul(out=px1[:], lhsT=mw0_t[:, 0, :], rhs=hgT[:, 0, :],
                             start=True, stop=False)
            nc.tensor.matmul(out=px1[:], lhsT=mw0_t[:, 1, :], rhs=hgT[:, 1, :],
                             start=False, stop=True)
            x1 = sb.tile([P, GPC], f32)
            nc.scalar.activation(out=x1[:], in_=px1[:],
                                 func=mybir.ActivationFunctionType.Relu,
                                 bias=mb0_t[:], scale=1.0)
            mw1_t = sb.tile([P, 64], f32)
            nc.sync.dma_start(out=mw1_t[:], in_=mw1[:])
            mb1_t = sb.tile([64, 1], f32)
            nc.sync.dma_start(out=mb1_t[:], in_=mb1[:])
            px2 = pp.tile([64, GPC], f32, space="PSUM", tag="w")
            nc.tensor.matmul(out=px2[:], lhsT=mw1_t[:], rhs=x1[:], start=True, stop=True)
            x2 = sb.tile([64, GPC], f32)
            nc.scalar.activation(out=x2[:], in_=px2[:],
                                 func=mybir.ActivationFunctionType.Relu,
                                 bias=mb1_t[:], scale=1.0)
            mw2_t = sb.tile([64, 2], f32)
            nc.sync.dma_start(out=mw2_t[:], in_=mw2[:])
            mb2_t = sb.tile([2, 1], f32)
            nc.sync.dma_start(out=mb2_t[:], in_=mb2[:])
            px3 = pp.tile([2, GPC], f32, space="PSUM", tag="w")
            nc.tensor.matmul(out=px3[:], lhsT=mw2_t[:], rhs=x2[:], start=True, stop=True)
            x3 = sb.tile([2, GPC], f32)
            nc.scalar.activation(out=x3[:], in_=px3[:],
                                 func=mybir.ActivationFunctionType.Identity,
                                 bias=mb2_t[:], scale=1.0)
            nc.sync.dma_start(out=out[:], in_=x3[:])

    return inp, out_t


_CACHE = {}


def _get_program(S):
    if S in _CACHE:
        return _CACHE[S]
    nc_b = bacc.Bacc("TRN2", target_bir_lowering=False, debug=False,
                     num_devices=NC)
    _build(nc_b, S)
    nc_b.compile()
    _CACHE[S] = nc_b
    return nc_b


def kernel(**inputs):
    cores, S = _host_prep(inputs)
    nc_b = _get_program(S)
    in_maps = [dict(c) for c in cores]
    res = run_bass_kernel_spmd(nc_b, in_maps, list(range(NC)))
    logits = np.zeros((G, 2), np.float32)
    for k in range(NC):
        lt = res.results[k]['logitsT']  # [2, GPC]
        logits[k * GPC:(k + 1) * GPC, :] = lt.T
    return logits

